# revision 1
# baseline (speedup 1.0000x reference)
"""Trainium2 Bass kernel for nn_Group_Attention (B=2, C=256, N=4096) on 8 NeuronCores.

Sharding: data-parallel over batch B (4 cores per sample); within a sample the
N (query-node) dimension is sharded 4-ways (1024 rows per core). Each core
computes its rows of the masked-softmax attention and the ResNet block;
InstanceNorm/BatchNorm statistics are combined with three small AllReduces.
"""

import numpy as np

import concourse.bass as bass  # noqa: F401
import concourse.tile as tile
import concourse.mybir as mybir
from concourse import bacc
from concourse.bass_isa import ReduceOp
from concourse.bass_utils import run_bass_kernel_spmd

f32 = mybir.dt.float32
f32r = mybir.dt.float32r
u8 = mybir.dt.uint8
AF = mybir.ActivationFunctionType
OP = mybir.AluOpType
AX = mybir.AxisListType

B, C, N = 2, 256, 4096
NCORES = 8
CPB = NCORES // B          # cores per batch sample
R = N // CPB               # query rows per core (1024)
NH = 2                     # n-halves per core
NF = R // NH               # 512 free-dim per chunk
MT = N // 128              # 32 m-tiles (key dim)
CT = C // 128              # 2 channel tiles
EPS = 1e-5
BIGNEG = 30000.0           # mask bias magnitude (exp(-30000) == 0 in f32)

_CACHED_NC = None


def build_nc():
    nc = bacc.Bacc("TRN2", target_bir_lowering=False, debug=False, num_devices=NCORES)

    # ---- per-core I/O ----
    x_d = nc.dram_tensor("x", [128, CT, N], f32r, kind="ExternalInput")
    xs_d = nc.dram_tensor("xs", [128, CT, R], f32, kind="ExternalInput")
    wT_d = nc.dram_tensor("wT", [128, CT, C], f32r, kind="ExternalInput")
    wrT_d = nc.dram_tensor("wrT", [128, CT, C], f32r, kind="ExternalInput")
    w1T_d = nc.dram_tensor("w1T", [128, CT, C], f32r, kind="ExternalInput")
    w2T_d = nc.dram_tensor("w2T", [128, CT, C], f32r, kind="ExternalInput")
    eaT_d = nc.dram_tensor("eaT", [N, R], f32, kind="ExternalInput")
    mkT_d = nc.dram_tensor("mkT", [N, R], u8, kind="ExternalInput")
    # packed per-channel params [128, CT, 9]:
    # 0:b_r 1:bn_r_w 2:bn_r_b 3:b1 4:bn1_w 5:bn1_b 6:b2 7:bn2_w 8:bn2_b
    prm_d = nc.dram_tensor("prm", [128, CT, 9], f32, kind="ExternalInput")
    bsel_d = nc.dram_tensor("bsel", [1, 2], f32, kind="ExternalInput")
    y_d = nc.dram_tensor("y", [128, CT, R], f32, kind="ExternalOutput")

    with tile.TileContext(nc) as tc:
        with (
            tc.tile_pool(name="const", bufs=1) as const,
            tc.tile_pool(name="stream", bufs=3) as stream,
            tc.tile_pool(name="stream2", bufs=2) as stream2,
            tc.tile_pool(name="small", bufs=1) as small,
            tc.tile_pool(name="psv", bufs=1, space="PSUM") as psv,
            tc.tile_pool(name="dram", bufs=1, space="DRAM") as dram,
        ):
            # ---------------- constants ----------------
            wT = const.tile([128, CT, C], f32r)
            wrT = const.tile([128, CT, C], f32r)
            w1T = const.tile([128, CT, C], f32r)
            w2T = const.tile([128, CT, C], f32r)
            prm = const.tile([128, CT, 9], f32)
            xs = const.tile([128, CT, R], f32)
            bsel = small.tile([1, 2], f32)
            nc.sync.dma_start(wT[:], wT_d[:])
            nc.sync.dma_start(wrT[:], wrT_d[:])
            nc.sync.dma_start(w1T[:], w1T_d[:])
            nc.sync.dma_start(w2T[:], w2T_d[:])
            nc.sync.dma_start(prm[:], prm_d[:])
            nc.sync.dma_start(xs[:], xs_d[:])
            nc.sync.dma_start(bsel[:], bsel_d[:])
            bselb = small.tile([128, 2], f32)
            nc.gpsimd.partition_broadcast(bselb[:], bsel[:])
            epsb = small.tile([128, 1], f32)
            nc.vector.memset(epsb[:], EPS)

            feat = const.tile([128, CT, N], f32r)   # q/4 for all keys (channel-major)
            featn = const.tile([128, CT, R], f32r)  # q/4 for my query rows
            vT = const.tile([128, MT, C + 1], f32r)  # elu(q)+1 with ones col
            valT = const.tile([128, CT, R], f32, tag="bigC", name="valT")
            nc.vector.memset(vT[:].bitcast(f32), 1.0)


            # ---------------- phase 0: feat / featT+elu ----------------
            XC = 2                  # x streamed in two m-chunks
            MX = N // XC            # 2048 keys per chunk
            with tc.tile_pool(name="xpool", bufs=1) as xpool:
                xsr = xpool.tile([128, CT, R], f32r)
                nc.vector.tensor_copy(xsr[:], xs[:])

                # featn: my query rows' q/4 from xs
                for ct in range(CT):
                    for nh in range(NH):
                        fp = psv.tile([128, NF], f32, tag="lp", name="fp")
                        for ci in range(CT):
                            nc.tensor.matmul(
                                fp[:],
                                wT[:, ci, ct * 128:(ct + 1) * 128],
                                xsr[:, ci, nh * NF:(nh + 1) * NF],
                                start=(ci == 0), stop=(ci == CT - 1),
                            )
                        nc.scalar.mul(featn[:, ct, nh * NF:(nh + 1) * NF], fp[:], 0.25)

                for xc in range(XC):
                    x_sb = xpool.tile([128, CT, MX], f32r, tag="xchunk", name="x_sb")
                    nc.sync.dma_start(x_sb[:], x_d[:, :, xc * MX:(xc + 1) * MX])
                    mbase = xc * MX

                    # feat[co, m] = 0.25 * W @ x (pre-scale so q.qT carries 1/16)
                    for ct in range(CT):
                        for mc in range(MX // NF):
                            fp = psv.tile([128, NF], f32, tag="lp", name="fp")
                            for ci in range(CT):
                                nc.tensor.matmul(
                                    fp[:],
                                    wT[:, ci, ct * 128:(ct + 1) * 128],
                                    x_sb[:, ci, mc * NF:(mc + 1) * NF],
                                    start=(ci == 0), stop=(ci == CT - 1),
                                )
                            nc.scalar.mul(
                                feat[:, ct, mbase + mc * NF:mbase + (mc + 1) * NF],
                                fp[:], 0.25,
                            )

                    # featT[m, c] then vT = exp(min(f,0)) + relu(f)  (elu + 1)
                    for mtl in range(MX // 128):
                        mt = xc * (MX // 128) + mtl
                        ftp = psv.tile([128, C], f32, tag="lp", name="ftp")
                        for ci in range(CT):
                            nc.tensor.matmul(
                                ftp[:],
                                x_sb[:, ci, mtl * 128:(mtl + 1) * 128],
                                wT[:, ci, :],
                                start=(ci == 0), stop=(ci == CT - 1),
                            )
                        t1 = stream2.tile([128, C], f32, tag="t")
                        nc.vector.tensor_scalar_min(t1[:], ftp[:], 0.0)
                        e1 = stream.tile([128, C], f32, tag="t2")
                        nc.scalar.activation(e1[:], t1[:], AF.Exp)
                        r1 = stream.tile([128, C], f32, tag="mkb")
                        nc.scalar.activation(r1[:], ftp[:], AF.Relu)
                        nc.vector.tensor_tensor(vT[:, mt, 0:C], e1[:], r1[:], OP.add)

            # ---------------- phase 1: attention ----------------
            # per n-half: accumulate valU[c, n] and rowsum over all m-tiles
            st_sum = small.tile([128, 2 * NH * CT], f32)
            st_sq = small.tile([128, 2 * NH * CT], f32)

            # val accumulators: [c-chunk, n-half] + packed rowsums (one bank)
            vp = [
                [
                    psv.tile([128, NF], f32, tag=f"vp{cb}_{nh}", name=f"vp{cb}_{nh}")
                    for nh in range(NH)
                ]
                for cb in range(CT)
            ]
            vpr = [
                psv.tile([1, NF], f32, tag=f"vpr_{nh}", name=f"vpr_{nh}")
                for nh in range(NH)
            ]

            for mt in range(MT):
                ea_t = stream2.tile([128, R], f32, tag="ea")
                mk_t = stream2.tile([128, R], u8, tag="mk")
                nc.sync.dma_start(ea_t[:], eaT_d[mt * 128:(mt + 1) * 128, :])
                nc.gpsimd.dma_start(mk_t[:], mkT_d[mt * 128:(mt + 1) * 128, :])

                lp = psv.tile([128, R], f32, tag="lp", name="lp")
                for nh in range(NH):
                    for ci in range(CT):
                        nc.tensor.matmul(
                            lp[:, nh * NF:(nh + 1) * NF],
                            feat[:, ci, mt * 128:(mt + 1) * 128],
                            featn[:, ci, nh * NF:(nh + 1) * NF],
                            start=(ci == 0), stop=(ci == CT - 1),
                        )
                t_t = stream2.tile([128, R], f32, tag="t")
                nc.vector.tensor_tensor(t_t[:], lp[:], ea_t[:], OP.mult)
                nc.scalar.activation(t_t[:], t_t[:], AF.Exp)
                e_t = stream2.tile([128, R], f32r, tag="e")
                nc.vector.tensor_tensor(e_t[:], t_t[:], mk_t[:], OP.mult)

                for nh in range(NH):
                    esl = e_t[:, nh * NF:(nh + 1) * NF]
                    for cb in range(CT):
                        nc.tensor.matmul(
                            vp[cb][nh][:], vT[:, mt, cb * 128:(cb + 1) * 128], esl,
                            start=(mt == 0), stop=(mt == MT - 1),
                        )
                    nc.tensor.matmul(
                        vpr[nh][:], vT[:, mt, C:C + 1], esl,
                        start=(mt == 0), stop=(mt == MT - 1),
                    )

            for nh in range(NH):
                nbase = nh * NF
                rs = small.tile([1, NF], f32, tag=f"rs{nh}", name=f"rs{nh}")
                nc.vector.reciprocal(rs[:], vpr[nh][:])
                rsb = small.tile([128, NF], f32, tag=f"rsb{nh}", name=f"rsb{nh}")
                nc.gpsimd.partition_broadcast(rsb[:], rs[:])
                for cb in range(CT):
                    dst = valT[:, cb, nbase:nbase + NF]
                    nc.vector.tensor_tensor(dst, vp[cb][nh][:], rsb[:], OP.mult)
                    col = nh * CT + cb
                    nc.vector.reduce_sum(st_sum[:, col:col + 1], dst, axis=AX.X)
                    sqj = stream2.tile([128, NF], f32, tag="e")
                    nc.scalar.activation(
                        sqj[:], dst, AF.Square, accum_out=st_sq[:, col:col + 1]
                    )

            # ---------------- AR1: val instance-norm stats ----------------
            stv = small.tile([128, 2], f32)
            nc.vector.reduce_sum(stv[:, 0:1], st_sum[:, 0:NH * CT], axis=AX.X)
            nc.vector.reduce_sum(stv[:, 1:2], st_sq[:, 0:NH * CT], axis=AX.X)
            stvr = small.tile([128, 2], f32)
            nc.gpsimd.partition_all_reduce(stvr[:], stv[:], 128, ReduceOp.add)

            ar1_in = small.tile([1, 8], f32)
            nc.vector.memset(ar1_in[:], 0.0)
            for b in range(B):
                nc.vector.tensor_scalar_mul(
                    ar1_in[0:1, 2 * b:2 * b + 2], stvr[0:1, 0:2], bsel[0:1, b:b + 1]
                )
            ar1_ind = dram.tile([1, 8], f32)
            ar1_outd = dram.tile([1, 8], f32)
            nc.sync.dma_start(ar1_ind[:], ar1_in[:])
            nc.gpsimd.collective_compute(
                "AllReduce", OP.add, replica_groups=[list(range(NCORES))],
                ins=[ar1_ind.opt()], outs=[ar1_outd.opt()],
            )
            ar1 = small.tile([1, 8], f32)
            nc.sync.dma_start(ar1[:], ar1_outd[:])

            # my-batch (sum, sq): sum over the two b-slots weighted by bsel
            msq = small.tile([1, 2], f32)
            tb0 = small.tile([1, 2], f32)
            tb1 = small.tile([1, 2], f32)
            nc.vector.tensor_scalar_mul(tb0[:], ar1[0:1, 0:2], bsel[0:1, 0:1])
            nc.vector.tensor_scalar_mul(tb1[:], ar1[0:1, 2:4], bsel[0:1, 1:2])
            nc.vector.tensor_tensor(msq[:], tb0[:], tb1[:], OP.add)
            # mu = sum/(N*C); var = sq/(N*C) - mu^2 ; rstd = 1/sqrt(var+eps)
            mu_v = small.tile([1, 1], f32)
            nc.vector.tensor_scalar_mul(mu_v[:], msq[0:1, 0:1], 1.0 / (N * C))
            e2_v = small.tile([1, 1], f32)
            nc.vector.tensor_scalar_mul(e2_v[:], msq[0:1, 1:2], 1.0 / (N * C))
            mu2_v = small.tile([1, 1], f32)
            nc.vector.tensor_tensor(mu2_v[:], mu_v[:], mu_v[:], OP.mult)
            var_v = small.tile([1, 1], f32)
            nc.vector.tensor_tensor(var_v[:], e2_v[:], mu2_v[:], OP.subtract)
            sd_v = small.tile([1, 1], f32)
            nc.scalar.activation(sd_v[:], var_v[:], AF.Sqrt, bias=epsb[0:1, :])
            rstd_v = small.tile([1, 1], f32)
            nc.vector.reciprocal(rstd_v[:], sd_v[:])
            murs = small.tile([1, 2], f32)
            nc.vector.tensor_copy(murs[0:1, 0:1], mu_v[:])
            nc.vector.tensor_copy(murs[0:1, 1:2], rstd_v[:])
            mursb = small.tile([128, 2], f32)
            nc.gpsimd.partition_broadcast(mursb[:], murs[:])

            # ---------------- feat_attn = IN(val) + x ----------------
            fa = const.tile([128, CT, R], f32r, tag="bigB", name="fa")
            for ct in range(CT):
                tmpn = stream.tile([128, R], f32, tag="big4k")
                nc.vector.tensor_scalar(
                    tmpn[:], valT[:, ct, :], mursb[:, 0:1], mursb[:, 1:2],
                    OP.subtract, OP.mult,
                )
                nc.vector.tensor_tensor(fa[:, ct, :], tmpn[:], xs[:, ct, :], OP.add)

            # ---------------- conv helper ----------------
            def conv(dst, w_sb, src, bias_idx, st_dst_sum=None, st_dst_sq=None):
                for ct in range(CT):
                    for nh in range(NH):
                        cp = psv.tile([128, NF], f32, tag="lp", name="cp")
                        for ci in range(CT):
                            nc.tensor.matmul(
                                cp[:],
                                w_sb[:, ci, ct * 128:(ct + 1) * 128],
                                src[:, ci, nh * NF:(nh + 1) * NF],
                                start=(ci == 0), stop=(ci == CT - 1),
                            )
                        nc.scalar.activation(
                            dst[:, ct, nh * NF:(nh + 1) * NF], cp[:],
                            AF.Identity, bias=prm[:, ct, bias_idx:bias_idx + 1],
                        )
                    if st_dst_sum is not None:
                        nc.vector.reduce_sum(st_dst_sum[:, ct:ct + 1], dst[:, ct, :], axis=AX.X)
                        sqj = stream.tile([128, R], f32, tag="big4k")
                        nc.scalar.activation(
                            sqj[:], dst[:, ct, :], AF.Square,
                            accum_out=st_dst_sq[:, ct:ct + 1],
                        )

            or_sb = const.tile([128, CT, R], f32)
            o1_sb = const.tile([128, CT, R], f32, tag="bigA", name="o1_sb")
            str_sum = small.tile([128, CT], f32)
            str_sq = small.tile([128, CT], f32)
            st1_sum = small.tile([128, CT], f32)
            st1_sq = small.tile([128, CT], f32)
            conv(or_sb, wrT, fa, 0, str_sum, str_sq)
            conv(o1_sb, w1T, fa, 3, st1_sum, st1_sq)

            # ---------------- AR2: conv_r (B,N) stats + conv1 per-(b,c) stats ----
            # payload [128, CT, 8]: 0 r_sum, 1 r_sq, 2-3 o1 b0 (sum, sq), 4-5 o1 b1, 6-7 pad
            ar2_in = small.tile([128, CT, 8], f32)
            nc.vector.memset(ar2_in[:], 0.0)
            for ct in range(CT):
                nc.vector.tensor_copy(ar2_in[:, ct, 0:1], str_sum[:, ct:ct + 1])
                nc.vector.tensor_copy(ar2_in[:, ct, 1:2], str_sq[:, ct:ct + 1])
                for b in range(B):
                    nc.vector.tensor_scalar_mul(
                        ar2_in[:, ct, 2 + 2 * b:3 + 2 * b], st1_sum[:, ct:ct + 1], bselb[:, b:b + 1]
                    )
                    nc.vector.tensor_scalar_mul(
                        ar2_in[:, ct, 3 + 2 * b:4 + 2 * b], st1_sq[:, ct:ct + 1], bselb[:, b:b + 1]
                    )
            ar2_ind = dram.tile([128, CT, 8], f32)
            ar2_outd = dram.tile([128, CT, 8], f32)
            nc.sync.dma_start(ar2_ind[:], ar2_in[:])
            nc.gpsimd.collective_compute(
                "AllReduce", OP.add, replica_groups=[list(range(NCORES))],
                ins=[ar2_ind.opt()], outs=[ar2_outd.opt()],
            )
            ar2 = small.tile([128, CT, 8], f32)
            nc.sync.dma_start(ar2[:], ar2_outd[:])

            # per-channel affine consts for bn_r and bn1(in1(.))
            alr = small.tile([128, CT], f32)   # bn_r scale
            ber = small.tile([128, CT], f32)   # bn_r bias
            al1 = small.tile([128, CT], f32)
            be1 = small.tile([128, CT], f32)

            def mkaffine_bn(al, be, sum_ap, sq_ap, cnt, w_ap, b_ap, ct):
                # mu = sum/cnt; var = sq/cnt - mu^2; al = w/sqrt(var+eps); be = b - mu*al
                mu = stream.tile([128, 1], f32, tag="aff_mu")
                nc.vector.tensor_scalar_mul(mu[:], sum_ap, 1.0 / cnt)
                e2 = stream.tile([128, 1], f32, tag="aff_e2")
                nc.vector.tensor_scalar_mul(e2[:], sq_ap, 1.0 / cnt)
                mu2 = stream.tile([128, 1], f32, tag="aff_mu2")
                nc.vector.tensor_tensor(mu2[:], mu[:], mu[:], OP.mult)
                var = stream.tile([128, 1], f32, tag="aff_var")
                nc.vector.tensor_tensor(var[:], e2[:], mu2[:], OP.subtract)
                sd = stream.tile([128, 1], f32, tag="aff_sd")
                nc.scalar.activation(sd[:], var[:], AF.Sqrt, bias=epsb[:])
                rstd = stream.tile([128, 1], f32, tag="aff_rstd")
                nc.vector.reciprocal(rstd[:], sd[:])
                nc.vector.tensor_tensor(al[:, ct:ct + 1], rstd[:], w_ap, OP.mult)
                t = stream.tile([128, 1], f32, tag="aff_t")
                nc.vector.tensor_tensor(t[:], mu[:], al[:, ct:ct + 1], OP.mult)
                nc.vector.tensor_tensor(be[:, ct:ct + 1], b_ap, t[:], OP.subtract)

            def mkaffine_in_bn(al, be, ar_sb, base, w_ap, b_ap, ct):
                # IN per (b,c) over N then BN over (B,N), composed:
                # v_b = sq_b/N - mu_b^2 ; varbn = mean_b[v_b/(v_b+eps)]
                # al = w * rstd_my * rsqrt(varbn+eps); be = b - mu_my*al
                vs = []
                mus = []
                for b in range(B):
                    mu = stream.tile([128, 1], f32, tag="ib_mu")
                    nc.vector.tensor_scalar_mul(mu[:], ar_sb[:, ct, base + 2 * b:base + 2 * b + 1], 1.0 / N)
                    e2 = stream.tile([128, 1], f32, tag="ib_e2")
                    nc.vector.tensor_scalar_mul(e2[:], ar_sb[:, ct, base + 2 * b + 1:base + 2 * b + 2], 1.0 / N)
                    mu2 = stream.tile([128, 1], f32, tag="ib_mu2")
                    nc.vector.tensor_tensor(mu2[:], mu[:], mu[:], OP.mult)
                    v = stream.tile([128, 1], f32, tag="ib_v")
                    nc.vector.tensor_tensor(v[:], e2[:], mu2[:], OP.subtract)
                    vs.append(v)
                    mus.append(mu)
                # varbn
                ratios = []
                for b in range(B):
                    vpe = stream.tile([128, 1], f32, tag="ib_vpe")
                    nc.vector.tensor_scalar_add(vpe[:], vs[b][:], EPS)
                    rv = stream.tile([128, 1], f32, tag="ib_rv")
                    nc.vector.reciprocal(rv[:], vpe[:])
                    rat = stream.tile([128, 1], f32, tag="ib_rat")
                    nc.vector.tensor_tensor(rat[:], vs[b][:], rv[:], OP.mult)
                    ratios.append(rat)
                varbn = stream.tile([128, 1], f32, tag="ib_varbn")
                nc.vector.tensor_tensor(varbn[:], ratios[0][:], ratios[1][:], OP.add)
                nc.vector.tensor_scalar_mul(varbn[:], varbn[:], 0.5)
                sdbn = stream.tile([128, 1], f32, tag="ib_sdbn")
                nc.scalar.activation(sdbn[:], varbn[:], AF.Sqrt, bias=epsb[:])
                rstdbn = stream.tile([128, 1], f32, tag="ib_rstdbn")
                nc.vector.reciprocal(rstdbn[:], sdbn[:])
                # select my batch's mu and v
                mu_my = stream.tile([128, 1], f32, tag="ib_mumy")
                v_my = stream.tile([128, 1], f32, tag="ib_vmy")
                ta = stream.tile([128, 1], f32, tag="ib_ta")
                nc.vector.tensor_scalar_mul(mu_my[:], mus[0][:], bselb[:, 0:1])
                nc.vector.tensor_scalar_mul(ta[:], mus[1][:], bselb[:, 1:2])
                nc.vector.tensor_tensor(mu_my[:], mu_my[:], ta[:], OP.add)
                nc.vector.tensor_scalar_mul(v_my[:], vs[0][:], bselb[:, 0:1])
                nc.vector.tensor_scalar_mul(ta[:], vs[1][:], bselb[:, 1:2])
                nc.vector.tensor_tensor(v_my[:], v_my[:], ta[:], OP.add)
                sd_my = stream.tile([128, 1], f32, tag="ib_sdmy")
                nc.scalar.activation(sd_my[:], v_my[:], AF.Sqrt, bias=epsb[:])
                rstd_my = stream.tile([128, 1], f32, tag="ib_rstdmy")
                nc.vector.reciprocal(rstd_my[:], sd_my[:])
                nc.vector.tensor_tensor(al[:, ct:ct + 1], rstd_my[:], rstdbn[:], OP.mult)
                nc.vector.tensor_tensor(al[:, ct:ct + 1], al[:, ct:ct + 1], w_ap, OP.mult)
                t = stream.tile([128, 1], f32, tag="ib_t")
                nc.vector.tensor_tensor(t[:], mu_my[:], al[:, ct:ct + 1], OP.mult)
                nc.vector.tensor_tensor(be[:, ct:ct + 1], b_ap, t[:], OP.subtract)

            for ct in range(CT):
                mkaffine_bn(
                    alr, ber, ar2[:, ct, 0:1], ar2[:, ct, 1:2], B * N,
                    prm[:, ct, 1:2], prm[:, ct, 2:3], ct,
                )
                mkaffine_in_bn(al1, be1, ar2, 2, prm[:, ct, 4:5], prm[:, ct, 5:6], ct)

            # h = relu(al1 * o1 + be1)
            h = const.tile([128, CT, R], f32r, tag="bigC", name="h")
            for ct in range(CT):
                for nh in range(NH):
                    nc.scalar.activation(
                        h[:, ct, nh * NF:(nh + 1) * NF],
                        o1_sb[:, ct, nh * NF:(nh + 1) * NF],
                        AF.Relu, bias=be1[:, ct:ct + 1], scale=al1[:, ct:ct + 1],
                    )

            # conv2 + stats
            o2_sb = const.tile([128, CT, R], f32, tag="bigB", name="o2_sb")
            st2_sum = small.tile([128, CT], f32)
            st2_sq = small.tile([128, CT], f32)
            conv(o2_sb, w2T, h, 6, st2_sum, st2_sq)

            # ---------------- AR3: conv2 per-(b,c) stats ----------------
            ar3_in = small.tile([128, CT, 4], f32)
            for ct in range(CT):
                for b in range(B):
                    nc.vector.tensor_scalar_mul(
                        ar3_in[:, ct, 2 * b:2 * b + 1], st2_sum[:, ct:ct + 1], bselb[:, b:b + 1]
                    )
                    nc.vector.tensor_scalar_mul(
                        ar3_in[:, ct, 2 * b + 1:2 * b + 2], st2_sq[:, ct:ct + 1], bselb[:, b:b + 1]
                    )
            ar3_ind = dram.tile([128, CT, 4], f32)
            ar3_outd = dram.tile([128, CT, 4], f32)
            nc.sync.dma_start(ar3_ind[:], ar3_in[:])
            nc.gpsimd.collective_compute(
                "AllReduce", OP.add, replica_groups=[list(range(NCORES))],
                ins=[ar3_ind.opt()], outs=[ar3_outd.opt()],
            )
            ar3 = small.tile([128, CT, 4], f32)
            nc.sync.dma_start(ar3[:], ar3_outd[:])

            al2 = small.tile([128, CT], f32)
            be2 = small.tile([128, CT], f32)
            for ct in range(CT):
                mkaffine_in_bn(al2, be2, ar3, 0, prm[:, ct, 7:8], prm[:, ct, 8:9], ct)

            # ---------------- final: y = relu(bn2(in2(o2)) + bn_r(or)) ----------
            y_sb = const.tile([128, CT, R], f32, tag="bigA", name="y_sb")
            for ct in range(CT):
                for nh in range(NH):
                    sl = slice(nh * NF, (nh + 1) * NF)
                    t1 = stream2.tile([128, NF], f32, tag="t")
                    nc.vector.tensor_scalar(
                        t1[:], o2_sb[:, ct, sl], al2[:, ct:ct + 1], be2[:, ct:ct + 1],
                        OP.mult, OP.add,
                    )
                    t2 = stream.tile([128, NF], f32, tag="t2")
                    nc.vector.tensor_scalar(
                        t2[:], or_sb[:, ct, sl], alr[:, ct:ct + 1], ber[:, ct:ct + 1],
                        OP.mult, OP.add,
                    )
                    t3 = stream.tile([128, NF], f32, tag="mkb")
                    nc.vector.tensor_tensor(t3[:], t1[:], t2[:], OP.add)
                    nc.scalar.activation(y_sb[:, ct, sl], t3[:], AF.Relu)
            nc.sync.dma_start(y_d[:], y_sb[:])

    nc.compile()
    return nc


def _prep_core_inputs(inputs):
    """Build the 8 per-core in_maps from the full problem inputs."""
    x = np.asarray(inputs["x"], dtype=np.float32)          # (B,C,N,1)
    edge_map = np.asarray(inputs["edge_map"])              # (B,N,N) int32
    edge_attr = np.asarray(inputs["edge_attr"], dtype=np.float32)

    def chan_major(w):  # (C, X) -> [128, CT, X]
        return np.ascontiguousarray(
            w.reshape(CT, 128, -1).transpose(1, 0, 2)
        ).astype(np.float32)

    wT = chan_major(np.asarray(inputs["linear_w"], dtype=np.float32).T)
    wrT = chan_major(np.asarray(inputs["w_r"], dtype=np.float32).T)
    w1T = chan_major(np.asarray(inputs["w1"], dtype=np.float32).T)
    w2T = chan_major(np.asarray(inputs["w2"], dtype=np.float32).T)

    pnames = ["b_r", "bn_r_w", "bn_r_b", "b1", "bn1_w", "bn1_b", "b2", "bn2_w", "bn2_b"]
    prm = np.stack(
        [np.asarray(inputs[p], dtype=np.float32).reshape(CT, 128) for p in pnames],
        axis=-1,
    ).transpose(1, 0, 2)  # [128, CT, 9]
    prm = np.ascontiguousarray(prm)

    in_maps = []
    for core in range(NCORES):
        b = core // CPB
        r0 = (core % CPB) * R
        xb = x[b, :, :, 0]                                  # (C, N)
        x_cm = chan_major(xb)                               # [128, CT, N]
        xs_cm = np.ascontiguousarray(x_cm[:, :, r0:r0 + R])
        eaT = np.ascontiguousarray(edge_attr[b].T[:, r0:r0 + R])   # (N, R)
        mkT = np.ascontiguousarray((edge_map[b].T[:, r0:r0 + R] != 0).astype(np.uint8))
        bsel = np.zeros((1, 2), np.float32)
        bsel[0, b] = 1.0
        in_maps.append({
            "x": x_cm, "xs": xs_cm, "wT": wT, "wrT": wrT, "w1T": w1T, "w2T": w2T,
            "eaT": eaT, "mkT": mkT, "prm": prm, "bsel": bsel,
        })
    return in_maps


def run(inputs, trace=False):
    global _CACHED_NC
    if _CACHED_NC is None:
        _CACHED_NC = build_nc()
    nc = _CACHED_NC
    in_maps = _prep_core_inputs(inputs)
    res = run_bass_kernel_spmd(
        nc, in_maps, core_ids=list(range(NCORES)), trace=trace
    )
    out = np.zeros((B, C, N, 1), np.float32)
    for core in range(NCORES):
        b = core // CPB
        r0 = (core % CPB) * R
        shard = res.results[core]["y"]                      # [128, CT, R]
        out[b, :, r0:r0 + R, 0] = shard.transpose(1, 0, 2).reshape(C, R)
    return out, res


def kernel(**inputs) -> np.ndarray:
    out, _ = run(inputs, trace=False)
    return out



# revision 6
# speedup vs baseline: 1.4428x; 1.4428x over previous
"""Trainium2 Bass kernel for nn_Group_Attention (B=2, C=256, N=4096) on 8 NeuronCores.

Sharding: data-parallel over batch B (4 cores per sample); within a sample the
N (query-node) dimension is sharded 4-ways (1024 rows per core).

v2 design:
- bf16 everywhere off-PSUM (halves DMA + unlocks DVE 2x/4x modes).
- mask folded into edge_attr on host (ea*m); masked logits recovered exactly
  via e = exp(t) - (t == 0).
- attention runs nh-outer so logits PSUM ping-pongs (2 banks) while the
  4 val accumulators + rowsum stay resident.
- conv_r / conv1 split linearly: o = rstd*A + B + c with A = W@val, B = W@x+b.
  Their BatchNorm/InstanceNorm moments are derived from (SA, SA2, SAB, SB, SB2)
  so the first two AllReduces of the baseline merge into one.
"""

import numpy as np
import ml_dtypes

import concourse.bass as bass  # noqa: F401
import concourse.tile as tile
import concourse.mybir as mybir
from concourse import bacc
from concourse.bass_isa import ReduceOp
from concourse.bass_utils import run_bass_kernel_spmd

f32 = mybir.dt.float32
bf16 = mybir.dt.bfloat16
AF = mybir.ActivationFunctionType
OP = mybir.AluOpType
AX = mybir.AxisListType
BF = ml_dtypes.bfloat16

B, C, N = 2, 256, 4096
NCORES = 8
CPB = NCORES // B          # cores per batch sample
R = N // CPB               # query rows per core (1024)
NH = 2                     # n-halves per core
NF = R // NH               # 512 free-dim per chunk
MT = N // 128              # 32 m-tiles (key dim)
CT = C // 128              # 2 channel tiles
XC = 2                     # x streamed in two chunks
MX = N // XC
EPS = 1e-5

_CACHED_NC = None


def build_nc():
    nc = bacc.Bacc("TRN2", target_bir_lowering=False, debug=False, num_devices=NCORES)

    # ---- per-core I/O ----
    x_d = nc.dram_tensor("x", [128, CT, N], bf16, kind="ExternalInput")
    xs_d = nc.dram_tensor("xs", [128, CT, R], bf16, kind="ExternalInput")
    wT_d = nc.dram_tensor("wT", [128, CT, C], bf16, kind="ExternalInput")
    wrT_d = nc.dram_tensor("wrT", [128, CT, C], bf16, kind="ExternalInput")
    w1T_d = nc.dram_tensor("w1T", [128, CT, C], bf16, kind="ExternalInput")
    w2T_d = nc.dram_tensor("w2T", [128, CT, C], bf16, kind="ExternalInput")
    ea_d = nc.dram_tensor("ea", [N, R], bf16, kind="ExternalInput")
    # packed per-channel params [128, CT, 11]:
    # 0:b_r 1:bn_r_w 2:bn_r_b 3:b1 4:bn1_w 5:bn1_b 6:b2 7:bn2_w 8:bn2_b 9:s_r 10:s1
    prm_d = nc.dram_tensor("prm", [128, CT, 11], f32, kind="ExternalInput")
    bsel_d = nc.dram_tensor("bsel", [1, 2], f32, kind="ExternalInput")
    y_d = nc.dram_tensor("y", [128, CT, R], f32, kind="ExternalOutput")

    with tile.TileContext(nc) as tc:
        with (
            tc.tile_pool(name="const", bufs=1) as const,
            tc.tile_pool(name="xpool", bufs=2) as xpool,
            tc.tile_pool(name="eapool", bufs=4) as eapool,
            tc.tile_pool(name="tpool", bufs=2) as tpool,
            tc.tile_pool(name="small", bufs=1) as small,
            tc.tile_pool(name="pslp", bufs=2, space="PSUM") as pslp,
            tc.tile_pool(name="pspp", bufs=3, space="PSUM") as pspp,
            tc.tile_pool(name="psv", bufs=1, space="PSUM") as psv,
            tc.tile_pool(name="dram", bufs=1, space="DRAM") as dram,
        ):
            # ---------------- constants ----------------
            wT = const.tile([128, CT, C], bf16)
            wrT = const.tile([128, CT, C], bf16)
            w1T = const.tile([128, CT, C], bf16)
            w2T = const.tile([128, CT, C], bf16)
            prm = const.tile([128, CT, 11], f32)
            xs = const.tile([128, CT, R], bf16)
            bsel = small.tile([1, 2], f32)
            nc.sync.dma_start(wT[:], wT_d[:])
            nc.sync.dma_start(wrT[:], wrT_d[:])
            nc.sync.dma_start(w1T[:], w1T_d[:])
            nc.sync.dma_start(w2T[:], w2T_d[:])
            nc.sync.dma_start(prm[:], prm_d[:])
            nc.sync.dma_start(xs[:], xs_d[:])
            nc.sync.dma_start(bsel[:], bsel_d[:])
            bselb = small.tile([128, 2], f32)
            nc.gpsimd.partition_broadcast(bselb[:], bsel[:])
            epsb = small.tile([128, 1], f32)
            nc.vector.memset(epsb[:], EPS)

            feat = const.tile([128, CT, N], bf16)    # q/4 for all keys (channel-major)
            featn = const.tile([128, CT, R], bf16)   # q/4 for my query rows
            vT = const.tile([128, MT, C + 1], bf16)  # elu(q)+1 with ones col
            valT = const.tile([128, CT, R], bf16)    # attn@v + 1 (channel-major)
            nc.vector.memset(vT[:], 1.0)

            A_r = const.tile([128, CT, R], bf16)     # W_r @ val
            A_1 = const.tile([128, CT, R], bf16)     # W_1 @ val
            B_r = const.tile([128, CT, R], bf16)     # W_r @ x + b_r (own rows)
            B_1 = const.tile([128, CT, R], bf16)     # W_1 @ x + b1

            # moment accumulators: slot k = moment + 5*nh
            # moments: 0:SA 1:SA2 2:SAB 3:SB 4:SB2
            mR = small.tile([128, CT, 10], f32)
            m1 = small.tile([128, CT, 10], f32)
            vmom = small.tile([128, 8], f32)         # val: sum slots 0-3, sq 4-7
            mo2 = small.tile([128, CT, 4], f32)      # o2: (sum, sq) x nh

            # ---------------- featn (own query rows) ----------------
            for ct in range(CT):
                for nh in range(NH):
                    fp = pspp.tile([128, NF], f32, tag="pp", name="fp")
                    for ci in range(CT):
                        nc.tensor.matmul(
                            fp[:],
                            wT[:, ci, ct * 128:(ct + 1) * 128],
                            xs[:, ci, nh * NF:(nh + 1) * NF],
                            start=(ci == 0), stop=(ci == CT - 1),
                        )
                    nc.vector.tensor_scalar_mul(
                        featn[:, ct, nh * NF:(nh + 1) * NF], fp[:], 0.25
                    )

            # ---------------- B_r / B_1 (own rows) + B moments ----------------
            for (wsb, Bdst, mX, bidx) in ((wrT, B_r, mR, 0), (w1T, B_1, m1, 3)):
                for ct in range(CT):
                    for nh in range(NH):
                        bp = pspp.tile([128, NF], f32, tag="pp", name="bp")
                        for ci in range(CT):
                            nc.tensor.matmul(
                                bp[:],
                                wsb[:, ci, ct * 128:(ct + 1) * 128],
                                xs[:, ci, nh * NF:(nh + 1) * NF],
                                start=(ci == 0), stop=(ci == CT - 1),
                            )
                        dst = Bdst[:, ct, nh * NF:(nh + 1) * NF]
                        nc.scalar.activation(
                            dst, bp[:], AF.Identity,
                            bias=prm[:, ct, bidx:bidx + 1],
                            accum_out=mX[:, ct, 3 + 5 * nh:4 + 5 * nh],
                        )
                        scr = tpool.tile([128, NF], bf16, tag="scr", name="scr")
                        nc.scalar.activation(
                            scr[:], dst, AF.Square,
                            accum_out=mX[:, ct, 4 + 5 * nh:5 + 5 * nh],
                        )

            # ---------------- feat / vT over x chunks ----------------
            for xc in range(XC):
                x_sb = xpool.tile([128, CT, MX], bf16, tag="xchunk", name="x_sb")
                nc.sync.dma_start(x_sb[:], x_d[:, :, xc * MX:(xc + 1) * MX])
                mbase = xc * MX

                # feat[co, m] = 0.25 * W @ x
                for ct in range(CT):
                    for mc in range(MX // NF):
                        fp = pspp.tile([128, NF], f32, tag="pp", name="fp2")
                        for ci in range(CT):
                            nc.tensor.matmul(
                                fp[:],
                                wT[:, ci, ct * 128:(ct + 1) * 128],
                                x_sb[:, ci, mc * NF:(mc + 1) * NF],
                                start=(ci == 0), stop=(ci == CT - 1),
                            )
                        nc.vector.tensor_scalar_mul(
                            feat[:, ct, mbase + mc * NF:mbase + (mc + 1) * NF],
                            fp[:], 0.25,
                        )

                # vT = elu(q)+1, two m-tiles per pass
                for pr in range(MX // 256):
                    ftp = pspp.tile([128, 2, C], f32, tag="pp", name="ftp")
                    for half in range(2):
                        mtl = pr * 2 + half
                        for ci in range(CT):
                            nc.tensor.matmul(
                                ftp[:, half, :],
                                x_sb[:, ci, mtl * 128:(mtl + 1) * 128],
                                wT[:, ci, :],
                                start=(ci == 0), stop=(ci == CT - 1),
                            )
                    mt0 = xc * (MX // 128) + pr * 2
                    t1 = tpool.tile([128, 2 * C], bf16, tag="t1", name="t1")
                    nc.vector.tensor_scalar_min(t1[:], ftp[:], 0.0)
                    e1 = tpool.tile([128, 2 * C], bf16, tag="e1", name="e1")
                    nc.scalar.activation(e1[:], t1[:], AF.Exp)
                    r1 = tpool.tile([128, 2 * C], bf16, tag="r1", name="r1")
                    nc.vector.tensor_scalar_max(r1[:], ftp[:], 0.0)
                    nc.vector.tensor_tensor(
                        vT[:, mt0:mt0 + 2, 0:C], e1[:], r1[:], OP.add
                    )

            # ---------------- attention (nh-outer) ----------------
            vp = [psv.tile([128, NF], f32, tag=f"vp{cb}", name=f"vp{cb}")
                  for cb in range(CT)]
            vpr = psv.tile([1, NF], f32, tag="vpr", name="vpr")

            for nh in range(NH):
                nbase = nh * NF
                for mt in range(MT):
                    ea_t = eapool.tile([128, NF], bf16, tag="ea", name="ea_t")
                    nc.sync.dma_start(
                        ea_t[:], ea_d[mt * 128:(mt + 1) * 128, nbase:nbase + NF]
                    )
                    lp = pslp.tile([128, NF], f32, tag="lp", name="lp")
                    for ci in range(CT):
                        nc.tensor.matmul(
                            lp[:],
                            feat[:, ci, mt * 128:(mt + 1) * 128],
                            featn[:, ci, nbase:nbase + NF],
                            start=(ci == 0), stop=(ci == CT - 1),
                        )
                    t_t = tpool.tile([128, NF], bf16, tag="t", name="t_t")
                    nc.vector.tensor_tensor(t_t[:], lp[:], ea_t[:], OP.mult)
                    e_t = tpool.tile([128, NF], bf16, tag="e", name="e_t")
                    nc.scalar.activation(e_t[:], t_t[:], AF.Exp)
                    z_t = tpool.tile([128, NF], bf16, tag="z", name="z_t")
                    nc.vector.tensor_scalar(z_t[:], t_t[:], 0.0, None, OP.is_equal)
                    e2_t = tpool.tile([128, NF], bf16, tag="e2", name="e2_t")
                    nc.gpsimd.tensor_tensor(e2_t[:], e_t[:], z_t[:], OP.subtract)
                    for cb in range(CT):
                        nc.tensor.matmul(
                            vp[cb][:], vT[:, mt, cb * 128:(cb + 1) * 128], e2_t[:],
                            start=(mt == 0), stop=(mt == MT - 1),
                        )
                    nc.tensor.matmul(
                        vpr[:], vT[:, mt, C:C + 1], e2_t[:],
                        start=(mt == 0), stop=(mt == MT - 1),
                    )

                # drain: normalize into valT + val moments
                rs = small.tile([1, NF], f32, tag=f"rs{nh}", name=f"rs{nh}")
                nc.vector.reciprocal(rs[:], vpr[:])
                rsb = small.tile([128, NF], f32, tag=f"rsb{nh}", name=f"rsb{nh}")
                nc.gpsimd.partition_broadcast(rsb[:], rs[:])
                for cb in range(CT):
                    dst = valT[:, cb, nbase:nbase + NF]
                    slot = nh * CT + cb
                    nc.vector.tensor_tensor(dst, vp[cb][:], rsb[:], OP.mult)
                    nc.vector.reduce_sum(
                        vmom[:, slot:slot + 1], dst, axis=AX.X
                    )
                    scr = tpool.tile([128, NF], bf16, tag="scr", name="sqv")
                    nc.scalar.activation(
                        scr[:], dst, AF.Square,
                        accum_out=vmom[:, 4 + slot:5 + slot],
                    )

                # A_r / A_1 for this nh + A moments
                for (wsb, Adst, Bsrc, mX) in (
                    (wrT, A_r, B_r, mR), (w1T, A_1, B_1, m1)
                ):
                    for ct in range(CT):
                        ap = pspp.tile([128, NF], f32, tag="pp", name="ap")
                        for ci in range(CT):
                            nc.tensor.matmul(
                                ap[:],
                                wsb[:, ci, ct * 128:(ct + 1) * 128],
                                valT[:, ci, nbase:nbase + NF],
                                start=(ci == 0), stop=(ci == CT - 1),
                            )
                        dstA = Adst[:, ct, nbase:nbase + NF]
                        nc.scalar.activation(
                            dstA, ap[:], AF.Identity,
                            accum_out=mX[:, ct, 0 + 5 * nh:1 + 5 * nh],
                        )
                        scr2 = tpool.tile([128, NF], bf16, tag="scr", name="sqa")
                        nc.scalar.activation(
                            scr2[:], dstA, AF.Square,
                            accum_out=mX[:, ct, 1 + 5 * nh:2 + 5 * nh],
                        )
                        scr3 = tpool.tile([128, NF], bf16, tag="scr", name="aba")
                        nc.vector.tensor_tensor(
                            scr3[:], dstA, Bsrc[:, ct, nbase:nbase + NF], OP.mult
                        )
                        nc.vector.reduce_sum(
                            mX[:, ct, 2 + 5 * nh:3 + 5 * nh], scr3[:], axis=AX.X
                        )

            # ---------------- AR1 payload ----------------
            stv = small.tile([128, 2], f32)
            nc.vector.reduce_sum(stv[:, 0:1], vmom[:, 0:4], axis=AX.X)
            nc.vector.reduce_sum(stv[:, 1:2], vmom[:, 4:8], axis=AX.X)
            stvr = small.tile([128, 2], f32)
            nc.gpsimd.partition_all_reduce(stvr[:], stv[:], 128, ReduceOp.add)

            cR = small.tile([128, CT, 5], f32)
            c1m = small.tile([128, CT, 5], f32)
            nc.vector.tensor_tensor(cR[:], mR[:, :, 0:5], mR[:, :, 5:10], OP.add)
            nc.vector.tensor_tensor(c1m[:], m1[:, :, 0:5], m1[:, :, 5:10], OP.add)

            pay = small.tile([128, CT, 24], f32)
            nc.vector.tensor_scalar_mul(pay[:, :, 0:5], cR[:], bselb[:, 0:1])
            nc.vector.tensor_scalar_mul(pay[:, :, 5:10], cR[:], bselb[:, 1:2])
            nc.vector.tensor_scalar_mul(pay[:, :, 10:15], c1m[:], bselb[:, 0:1])
            nc.vector.tensor_scalar_mul(pay[:, :, 15:20], c1m[:], bselb[:, 1:2])
            for ct in range(CT):
                nc.vector.tensor_scalar_mul(
                    pay[:, ct, 20:22], stvr[:], bselb[:, 0:1]
                )
                nc.vector.tensor_scalar_mul(
                    pay[:, ct, 22:24], stvr[:], bselb[:, 1:2]
                )

            ar1_ind = dram.tile([128, CT, 24], f32)
            ar1_outd = dram.tile([128, CT, 24], f32)
            nc.sync.dma_start(ar1_ind[:], pay[:])
            nc.gpsimd.collective_compute(
                "AllReduce", OP.add, replica_groups=[list(range(NCORES))],
                ins=[ar1_ind.opt()], outs=[ar1_outd.opt()],
            )
            par = small.tile([128, CT, 24], f32)
            nc.sync.dma_start(par[:], ar1_outd[:])

            # ---------------- post-AR1 channel math ----------------
            V, S = nc.vector, nc.scalar

            def newt(shape, name):
                return small.tile(shape, f32, tag=name, name=name)

            # per-batch val IN stats: mu_b, r_b (rstd), r2_b
            mu_b, r_b, r2_b = [], [], []
            for b in range(B):
                mu = newt([128, 1], f"vmu{b}")
                V.tensor_scalar_mul(mu[:], par[:, 0, 20 + 2 * b:21 + 2 * b], 1.0 / (N * C))
                e2v = newt([128, 1], f"ve2{b}")
                V.tensor_scalar_mul(e2v[:], par[:, 0, 21 + 2 * b:22 + 2 * b], 1.0 / (N * C))
                mu2 = newt([128, 1], f"vmu2{b}")
                V.tensor_tensor(mu2[:], mu[:], mu[:], OP.mult)
                var = newt([128, 1], f"vvar{b}")
                V.tensor_tensor(var[:], e2v[:], mu2[:], OP.subtract)
                sd = newt([128, 1], f"vsd{b}")
                S.activation(sd[:], var[:], AF.Sqrt, bias=epsb[:])
                r = newt([128, 1], f"vr{b}")
                V.reciprocal(r[:], sd[:])
                r2 = newt([128, 1], f"vr2{b}")
                V.tensor_tensor(r2[:], r[:], r[:], OP.mult)
                mu_b.append(mu); r_b.append(r); r2_b.append(r2)

            def bmix(name, parts):
                # my-batch selection: sum_b parts[b]*bsel[b]
                o = newt([128, 1], name)
                t = newt([128, 1], name + "_t")
                V.tensor_scalar_mul(o[:], parts[0][:], bselb[:, 0:1])
                V.tensor_scalar_mul(t[:], parts[1][:], bselb[:, 1:2])
                V.tensor_tensor(o[:], o[:], t[:], OP.add)
                return o

            r_my = bmix("r_my", r_b)

            def conv_moments(base, s_slot, name):
                """Per-batch (So, Sq, cc, w) from payload moments at `base`.

                o = r_b*A + B + cc_b ; cc_b = -mu_b*r_b*s
                So_b = r_b*SA + SB + N*cc_b
                Sq_b = r2_b*SA2 + 2 r_b*SAB + 2 cc_b*(r_b*SA + SB) + SB2 + N*cc_b^2
                """
                So, Sq, cc = [], [], []
                s_ap = prm[:, :, s_slot:s_slot + 1]
                for b in range(B):
                    o = base + 5 * b
                    SA = par[:, :, o:o + 1]
                    SA2 = par[:, :, o + 1:o + 2]
                    SAB = par[:, :, o + 2:o + 3]
                    SB = par[:, :, o + 3:o + 4]
                    SB2 = par[:, :, o + 4:o + 5]
                    nmr = newt([128, 1], f"{name}nmr{b}")
                    V.tensor_tensor(nmr[:], mu_b[b][:], r_b[b][:], OP.mult)
                    V.tensor_scalar_mul(nmr[:], nmr[:], -1.0)
                    ccb = newt([128, CT, 1], f"{name}cc{b}")
                    V.tensor_scalar_mul(ccb[:], s_ap, nmr[:])
                    w = newt([128, CT, 1], f"{name}w{b}")
                    V.tensor_scalar_mul(w[:], SA, r_b[b][:])
                    V.tensor_tensor(w[:], w[:], SB, OP.add)
                    so = newt([128, CT, 1], f"{name}so{b}")
                    V.tensor_scalar_mul(so[:], ccb[:], float(N))
                    V.tensor_tensor(so[:], so[:], w[:], OP.add)
                    q1 = newt([128, CT, 1], f"{name}q1{b}")
                    V.tensor_scalar_mul(q1[:], SA2, r2_b[b][:])
                    q2 = newt([128, CT, 1], f"{name}q2{b}")
                    V.tensor_scalar_mul(q2[:], SAB, r_b[b][:])
                    V.tensor_scalar_mul(q2[:], q2[:], 2.0)
                    q3 = newt([128, CT, 1], f"{name}q3{b}")
                    V.tensor_tensor(q3[:], ccb[:], w[:], OP.mult)
                    V.tensor_scalar_mul(q3[:], q3[:], 2.0)
                    q4 = newt([128, CT, 1], f"{name}q4{b}")
                    V.tensor_tensor(q4[:], ccb[:], ccb[:], OP.mult)
                    V.tensor_scalar_mul(q4[:], q4[:], float(N))
                    sq = newt([128, CT, 1], f"{name}sq{b}")
                    V.tensor_tensor(sq[:], q1[:], q2[:], OP.add)
                    V.tensor_tensor(sq[:], sq[:], q3[:], OP.add)
                    V.tensor_tensor(sq[:], sq[:], q4[:], OP.add)
                    V.tensor_tensor(sq[:], sq[:], SB2, OP.add)
                    So.append(so); Sq.append(sq); cc.append(ccb)
                return So, Sq, cc

            def cmix(name, parts):
                # my-batch selection for [128, CT, 1] tiles
                o = newt([128, CT, 1], name)
                t = newt([128, CT, 1], name + "_t")
                V.tensor_scalar_mul(o[:], parts[0][:], bselb[:, 0:1])
                V.tensor_scalar_mul(t[:], parts[1][:], bselb[:, 1:2])
                V.tensor_tensor(o[:], o[:], t[:], OP.add)
                return o

            def rsqrt_eps(src, name):
                sd = newt(list(src.shape), name + "_sd")
                S.activation(sd[:], src[:], AF.Sqrt, bias=epsb[:])
                r = newt(list(src.shape), name + "_r")
                V.reciprocal(r[:], sd[:])
                return r

            # ---- conv_r: BN over (B,N) ----
            So_r, Sq_r, cc_r = conv_moments(0, 9, "cr")
            mu_r = newt([128, CT, 1], "mu_r")
            V.tensor_tensor(mu_r[:], So_r[0][:], So_r[1][:], OP.add)
            V.tensor_scalar_mul(mu_r[:], mu_r[:], 1.0 / (B * N))
            eq_r = newt([128, CT, 1], "eq_r")
            V.tensor_tensor(eq_r[:], Sq_r[0][:], Sq_r[1][:], OP.add)
            V.tensor_scalar_mul(eq_r[:], eq_r[:], 1.0 / (B * N))
            mr2 = newt([128, CT, 1], "mr2")
            V.tensor_tensor(mr2[:], mu_r[:], mu_r[:], OP.mult)
            var_r = newt([128, CT, 1], "var_r")
            V.tensor_tensor(var_r[:], eq_r[:], mr2[:], OP.subtract)
            rr = rsqrt_eps(var_r, "rr")
            al_r = newt([128, CT, 1], "al_r")
            V.tensor_tensor(al_r[:], rr[:], prm[:, :, 1:2], OP.mult)
            be_r = newt([128, CT, 1], "be_r")
            V.tensor_tensor(be_r[:], mu_r[:], al_r[:], OP.mult)
            V.tensor_tensor(be_r[:], prm[:, :, 2:3], be_r[:], OP.subtract)
            cc_rmy = cmix("cc_rmy", cc_r)
            gA_r = newt([128, CT, 1], "gA_r")
            V.tensor_scalar_mul(gA_r[:], al_r[:], r_my[:])
            g0_r = newt([128, CT, 1], "g0_r")
            V.tensor_tensor(g0_r[:], al_r[:], cc_rmy[:], OP.mult)
            V.tensor_tensor(g0_r[:], g0_r[:], be_r[:], OP.add)

            # ---- conv1: IN per (b,c) then BN ----
            So_1, Sq_1, cc_1 = conv_moments(10, 10, "c1")
            mu1, rin1, rat1 = [], [], []
            for b in range(B):
                m_ = newt([128, CT, 1], f"c1mu{b}")
                V.tensor_scalar_mul(m_[:], So_1[b][:], 1.0 / N)
                e_ = newt([128, CT, 1], f"c1e{b}")
                V.tensor_scalar_mul(e_[:], Sq_1[b][:], 1.0 / N)
                m2_ = newt([128, CT, 1], f"c1m2{b}")
                V.tensor_tensor(m2_[:], m_[:], m_[:], OP.mult)
                v_ = newt([128, CT, 1], f"c1v{b}")
                V.tensor_tensor(v_[:], e_[:], m2_[:], OP.subtract)
                rin = rsqrt_eps(v_, f"c1rin{b}")
                rin2 = newt([128, CT, 1], f"c1rin2{b}")
                V.tensor_tensor(rin2[:], rin[:], rin[:], OP.mult)
                rat = newt([128, CT, 1], f"c1rat{b}")
                V.tensor_tensor(rat[:], v_[:], rin2[:], OP.mult)
                mu1.append(m_); rin1.append(rin); rat1.append(rat)
            varbn1 = newt([128, CT, 1], "varbn1")
            V.tensor_tensor(varbn1[:], rat1[0][:], rat1[1][:], OP.add)
            V.tensor_scalar_mul(varbn1[:], varbn1[:], 0.5)
            rbn1 = rsqrt_eps(varbn1, "rbn1")
            mu1_my = cmix("mu1_my", mu1)
            rin1_my = cmix("rin1_my", rin1)
            cc1_my = cmix("cc1_my", cc_1)
            al1 = newt([128, CT, 1], "al1")
            V.tensor_tensor(al1[:], rin1_my[:], rbn1[:], OP.mult)
            V.tensor_tensor(al1[:], al1[:], prm[:, :, 4:5], OP.mult)
            gA1 = newt([128, CT, 1], "gA1")
            V.tensor_scalar_mul(gA1[:], al1[:], r_my[:])
            g01 = newt([128, CT, 1], "g01")
            V.tensor_tensor(g01[:], cc1_my[:], mu1_my[:], OP.subtract)
            V.tensor_tensor(g01[:], g01[:], al1[:], OP.mult)
            V.tensor_tensor(g01[:], g01[:], prm[:, :, 5:6], OP.add)

            # ---------------- h = relu(gA1*A1 + al1*B1 + g01) ----------------
            h = const.tile([128, CT, R], bf16)
            orr = const.tile([128, CT, R], bf16)     # al_r*conv_r(fa) + be_r
            o2 = const.tile([128, CT, R], bf16)
            for ct in range(CT):
                for nh in range(NH):
                    sl = slice(nh * NF, (nh + 1) * NF)
                    ta = tpool.tile([128, NF], bf16, tag="as1", name="ta")
                    V.tensor_scalar(
                        ta[:], A_1[:, ct, sl], gA1[:, ct, 0:1], g01[:, ct, 0:1],
                        OP.mult, OP.add,
                    )
                    tb = tpool.tile([128, NF], bf16, tag="as2", name="tb")
                    V.tensor_scalar_mul(tb[:], B_1[:, ct, sl], al1[:, ct, 0:1])
                    tcs = tpool.tile([128, NF], bf16, tag="as3", name="tc")
                    V.tensor_tensor(tcs[:], ta[:], tb[:], OP.add)
                    V.tensor_scalar_max(h[:, ct, sl], tcs[:], 0.0)

            # orr = gA_r*A_r + al_r*B_r + g0_r (overlaps with conv2)
            for ct in range(CT):
                for nh in range(NH):
                    sl = slice(nh * NF, (nh + 1) * NF)
                    ta = tpool.tile([128, NF], bf16, tag="as1", name="ta2")
                    V.tensor_scalar(
                        ta[:], A_r[:, ct, sl], gA_r[:, ct, 0:1], g0_r[:, ct, 0:1],
                        OP.mult, OP.add,
                    )
                    tb = tpool.tile([128, NF], bf16, tag="as2", name="tb2")
                    V.tensor_scalar_mul(tb[:], B_r[:, ct, sl], al_r[:, ct, 0:1])
                    V.tensor_tensor(orr[:, ct, sl], ta[:], tb[:], OP.add)

            # ---------------- conv2 + stats ----------------
            for ct in range(CT):
                for nh in range(NH):
                    sl = slice(nh * NF, (nh + 1) * NF)
                    cp = pspp.tile([128, NF], f32, tag="pp", name="cp")
                    for ci in range(CT):
                        nc.tensor.matmul(
                            cp[:],
                            w2T[:, ci, ct * 128:(ct + 1) * 128],
                            h[:, ci, sl],
                            start=(ci == 0), stop=(ci == CT - 1),
                        )
                    nc.scalar.activation(
                        o2[:, ct, sl], cp[:], AF.Identity,
                        bias=prm[:, ct, 6:7],
                        accum_out=mo2[:, ct, 0 + 2 * nh:1 + 2 * nh],
                    )
                    scr = tpool.tile([128, NF], bf16, tag="scr", name="sq2")
                    nc.scalar.activation(
                        scr[:], o2[:, ct, sl], AF.Square,
                        accum_out=mo2[:, ct, 1 + 2 * nh:2 + 2 * nh],
                    )

            # ---------------- AR3 ----------------
            c2m = small.tile([128, CT, 2], f32)
            nc.vector.tensor_tensor(c2m[:], mo2[:, :, 0:2], mo2[:, :, 2:4], OP.add)
            pay3 = small.tile([128, CT, 4], f32)
            V.tensor_scalar_mul(pay3[:, :, 0:2], c2m[:], bselb[:, 0:1])
            V.tensor_scalar_mul(pay3[:, :, 2:4], c2m[:], bselb[:, 1:2])
            ar3_ind = dram.tile([128, CT, 4], f32)
            ar3_outd = dram.tile([128, CT, 4], f32)
            nc.sync.dma_start(ar3_ind[:], pay3[:])
            nc.gpsimd.collective_compute(
                "AllReduce", OP.add, replica_groups=[list(range(NCORES))],
                ins=[ar3_ind.opt()], outs=[ar3_outd.opt()],
            )
            par3 = small.tile([128, CT, 4], f32)
            nc.sync.dma_start(par3[:], ar3_outd[:])

            mu2l, rin2l, rat2l = [], [], []
            for b in range(B):
                m_ = newt([128, CT, 1], f"c2mu{b}")
                V.tensor_scalar_mul(m_[:], par3[:, :, 2 * b:2 * b + 1], 1.0 / N)
                e_ = newt([128, CT, 1], f"c2e{b}")
                V.tensor_scalar_mul(e_[:], par3[:, :, 2 * b + 1:2 * b + 2], 1.0 / N)
                m2_ = newt([128, CT, 1], f"c2m2{b}")
                V.tensor_tensor(m2_[:], m_[:], m_[:], OP.mult)
                v_ = newt([128, CT, 1], f"c2v{b}")
                V.tensor_tensor(v_[:], e_[:], m2_[:], OP.subtract)
                rin = rsqrt_eps(v_, f"c2rin{b}")
                rin2 = newt([128, CT, 1], f"c2rin2{b}")
                V.tensor_tensor(rin2[:], rin[:], rin[:], OP.mult)
                rat = newt([128, CT, 1], f"c2rat{b}")
                V.tensor_tensor(rat[:], v_[:], rin2[:], OP.mult)
                mu2l.append(m_); rin2l.append(rin); rat2l.append(rat)
            varbn2 = newt([128, CT, 1], "varbn2")
            V.tensor_tensor(varbn2[:], rat2l[0][:], rat2l[1][:], OP.add)
            V.tensor_scalar_mul(varbn2[:], varbn2[:], 0.5)
            rbn2 = rsqrt_eps(varbn2, "rbn2")
            mu2_my = cmix("mu2_my", mu2l)
            rin2_my = cmix("rin2_my", rin2l)
            al2 = newt([128, CT, 1], "al2")
            V.tensor_tensor(al2[:], rin2_my[:], rbn2[:], OP.mult)
            V.tensor_tensor(al2[:], al2[:], prm[:, :, 7:8], OP.mult)
            g02 = newt([128, CT, 1], "g02")
            V.tensor_tensor(g02[:], mu2_my[:], al2[:], OP.mult)
            V.tensor_tensor(g02[:], prm[:, :, 8:9], g02[:], OP.subtract)

            # ---------------- final: y = relu(al2*o2 + g02 + orr) ----------------
            y_sb = const.tile([128, CT, R], f32)
            for ct in range(CT):
                for nh in range(NH):
                    sl = slice(nh * NF, (nh + 1) * NF)
                    f1 = tpool.tile([128, NF], bf16, tag="as1", name="f1")
                    V.tensor_scalar(
                        f1[:], o2[:, ct, sl], al2[:, ct, 0:1], g02[:, ct, 0:1],
                        OP.mult, OP.add,
                    )
                    f2 = tpool.tile([128, NF], bf16, tag="as2", name="f2")
                    V.tensor_tensor(f2[:], f1[:], orr[:, ct, sl], OP.add)
                    V.tensor_scalar_max(y_sb[:, ct, sl], f2[:], 0.0)
            nc.sync.dma_start(y_d[:], y_sb[:])

    nc.compile()
    return nc


def _prep_core_inputs(inputs):
    """Build the 8 per-core in_maps from the full problem inputs."""
    x = np.asarray(inputs["x"], dtype=np.float32)[:, :, :, 0]   # (B,C,N)
    edge_map = np.asarray(inputs["edge_map"])                   # (B,N,N) int32
    edge_attr = np.asarray(inputs["edge_attr"], dtype=np.float32)
    eam = (edge_attr * (edge_map != 0)).astype(BF)              # (B,N,N) bf16

    def chan_major(w, dt):  # (C, X) -> [128, CT, X]
        return np.ascontiguousarray(
            w.reshape(CT, 128, -1).transpose(1, 0, 2)
        ).astype(dt)

    wT = chan_major(np.asarray(inputs["linear_w"], dtype=np.float32).T, BF)
    wrT = chan_major(np.asarray(inputs["w_r"], dtype=np.float32).T, BF)
    w1T = chan_major(np.asarray(inputs["w1"], dtype=np.float32).T, BF)
    w2T = chan_major(np.asarray(inputs["w2"], dtype=np.float32).T, BF)

    s_r = np.asarray(inputs["w_r"], dtype=np.float32).sum(axis=1)
    s1 = np.asarray(inputs["w1"], dtype=np.float32).sum(axis=1)
    pnames = ["b_r", "bn_r_w", "bn_r_b", "b1", "bn1_w", "bn1_b", "b2", "bn2_w", "bn2_b"]
    pcols = [np.asarray(inputs[p], dtype=np.float32) for p in pnames] + [s_r, s1]
    prm = np.stack([p.reshape(CT, 128) for p in pcols], axis=-1).transpose(1, 0, 2)
    prm = np.ascontiguousarray(prm).astype(np.float32)          # [128, CT, 11]

    in_maps = []
    for core in range(NCORES):
        b = core // CPB
        r0 = (core % CPB) * R
        x_cm = chan_major(x[b], BF)                             # [128, CT, N]
        xs_cm = np.ascontiguousarray(x_cm[:, :, r0:r0 + R])
        ea_core = np.ascontiguousarray(eam[b].T[:, r0:r0 + R])  # (N, R) bf16
        bsel = np.zeros((1, 2), np.float32)
        bsel[0, b] = 1.0
        in_maps.append({
            "x": x_cm, "xs": xs_cm, "wT": wT, "wrT": wrT, "w1T": w1T, "w2T": w2T,
            "ea": ea_core, "prm": prm, "bsel": bsel,
        })
    return in_maps


def run(inputs, trace=False):
    global _CACHED_NC
    if _CACHED_NC is None:
        _CACHED_NC = build_nc()
    nc = _CACHED_NC
    in_maps = _prep_core_inputs(inputs)
    res = run_bass_kernel_spmd(
        nc, in_maps, core_ids=list(range(NCORES)), trace=trace
    )
    out = np.zeros((B, C, N, 1), np.float32)
    for core in range(NCORES):
        b = core // CPB
        r0 = (core % CPB) * R
        shard = res.results[core]["y"]                          # [128, CT, R]
        out[b, :, r0:r0 + R, 0] = shard.transpose(1, 0, 2).reshape(C, R)
    return out, res


def kernel(**inputs) -> np.ndarray:
    out, _ = run(inputs, trace=False)
    return out
